# revision 19
# baseline (speedup 1.0000x reference)
"""ConformerDecoder Trainium2 Bass kernel (v2).

Sharding: pure data-parallel over batch B=8 -> one sample per NeuronCore.

Activations live in "transposed" layout [feature-on-partitions, T-free] so the
whole matmul chain runs with weights as natural lhsT operands, bf16 matmuls
with fp32 PSUM accumulation.

v2 changes vs baseline (3.37ms -> target ~1.3ms):
- Attention windows shrunk 384 -> 192 key-columns (W=64 band fits in a
  32-shifted 192 window): halves scores/exp/mask/AV work on every engine.
  AV uses partition-offset sub-matmuls against the natural V tiles, so no
  shifted V copy is needed; edge blocks simply skip the out-of-range pieces.
- Depthwise conv K=31 split across engines: 16 even taps as fp16
  scalar_tensor_tensor acc-chains on DVE (4x perf mode: 16-bit, SBUF-only,
  4B-aligned via even shifts), 15 odd taps on PE as diag-strip matmuls
  (strips built by the ACT engine, not DVE), combined during the PSUM
  evacuation.  No more 31x128-wasteful all-PE conv and no DVE strip stall.
- LayerNorm rstd via the fp16 Quake bit-trick (int16 shift/xor seed + 2
  inline Newton steps, all 4x-mode DVE ops) instead of slow f32 Newton
  chains; mean/sumsq scaled copies moved to ACT.  GroupNorm rstd same trick
  in f32 [P,1] (replaces a 14-iteration Newton chain that stalled PE).
- GroupNorm affine+SiLU fused into a single per-chunk ACT op (scale/bias are
  per-partition APs).
- Engine rebalance: psum evacuations split between ACT and DVE.
"""

import os
import sys
from contextlib import ExitStack

for _p in ("/opt/trn_rl_repo",):
    if _p not in sys.path:
        sys.path.insert(0, _p)

import numpy as np
import ml_dtypes

import concourse.bass as bass
import concourse.tile as tile
from concourse import bacc
from concourse import mybir
from concourse.bass_utils import run_bass_kernel_spmd

BF16 = mybir.dt.bfloat16
FP16 = mybir.dt.float16
I16 = mybir.dt.int16
I32 = mybir.dt.int32
F32 = mybir.dt.float32
AF = mybir.ActivationFunctionType
OP = mybir.AluOpType

L, D, H, T, B = 4, 512, 8, 1024, 8
FF = 4 * D            # 2048
EC = 2 * D            # 1024 conv channels
KK = 31               # conv kernel size
WIN = 64              # attention window
DH = D // H           # 64
P = 128
DC = D // P           # 4 feature chunks
FC = FF // P          # 16
CC = EC // P          # 8
TB = T // P           # 8 token blocks
NT = 512              # matmul moving free dim
TC = T // NT          # 2 t-columns
KW = 192              # attention window width (32-shifted)
EPS = 1e-5

# bf16 / fp32 rsqrt bit-trick magics
MAGICBF = 0x5F34
MAGIC32 = 0x5F3759DF

TRACE = False          # set by test.py for profiling runs
TRACE_KW = {}
LAST_RESULT = None     # BassKernelResults of last run (read by test.py)
LAYERS = int(os.environ.get("CONF_LAYERS", str(L)))
PHASES = os.environ.get("CONF_PHASES", "fac2b")

# bias row indices in the packed bias tensor
BR_F1B1, BR_F1B2, BR_Q, BR_K, BR_V, BR_O, BR_P1, BR_P2, BR_F2B1, BR_F2B2 = range(10)

DVE_TAPS = tuple(range(0, KK, 2))   # 16 even taps (4B-aligned fp16 reads)
PE_TAPS = tuple(range(1, KK, 2))    # 15 odd taps (strip matmuls)


def _band_masks():
    """[128, KW] multiplicative masks (mid, qb0, qb7) in bf16.

    Window column c for q-block qb is absolute key k = qb*128 - 32 + c;
    row i is query q = qb*128 + i.  Valid iff |q-k| <= WIN/2 and 0<=k<T,
    i.e. c in [i, i+64], plus c>=32 for qb=0 and c<=159 for qb=7.
    """
    i = np.arange(P)[:, None]
    c = np.arange(KW)[None, :]
    band = np.abs(i + 32 - c) <= WIN // 2
    q0 = band & (c >= 32)
    q7 = band & (c <= 159)
    to = lambda m: np.ascontiguousarray(m.astype(ml_dtypes.bfloat16))
    return to(band), to(q0), to(q7)


def build_program(flags):
    ln_gen = not flags["ln_trivial"]
    bias_gen = not flags["bias_trivial"]
    fin_gen = not flags["final_trivial"]
    dwb_gen = not flags["dwb_trivial"]

    nc = bacc.Bacc("TRN2", target_bir_lowering=False, debug=False)

    xt_d = nc.dram_tensor("x_t", [D, T], F32, kind="ExternalInput").ap()
    out_d = nc.dram_tensor("out_t", [D, T], F32, kind="ExternalOutput").ap()

    def win(name, shape):
        return nc.dram_tensor(name, shape, BF16, kind="ExternalInput").ap()

    w_f1a = win("f1w1", [L, D, FF])
    w_f1b = win("f1w2", [L, FF, D])
    w_f2a = win("f2w1", [L, D, FF])
    w_f2b = win("f2w2", [L, FF, D])
    w_q = win("wq", [L, D, D])
    w_kk = win("wk", [L, D, D])
    w_v = win("wv", [L, D, D])
    w_o = win("wo", [L, D, D])
    w_p1 = win("pw1", [L, D, 2 * EC])
    w_p2 = win("pw2", [L, EC, D])
    w_dw = nc.dram_tensor("dw", [L, P, CC, KK], F32, kind="ExternalInput").ap()
    w_strip = nc.dram_tensor("strips", [L, CC, P, len(PE_TAPS), P], FP16,
                             kind="ExternalInput").ap()
    w_gn = nc.dram_tensor("gn_aff", [L, 2, EC], F32, kind="ExternalInput").ap()
    w_gains = (nc.dram_tensor("ln_gains", [L, 10, D], F32, kind="ExternalInput").ap()
                if ln_gen else None)
    w_fin = (nc.dram_tensor("final_aff", [2, D], F32, kind="ExternalInput").ap()
             if fin_gen else None)
    w_bias = win("biases", [L, 10, 2 * EC]) if bias_gen else None
    w_dwb = (nc.dram_tensor("dwb", [L, P, CC], F32, kind="ExternalInput").ap()
             if dwb_gen else None)

    band_mid, band_q0, band_q7 = _band_masks()
    bmid_d = nc.inline_tensor(band_mid, "band_mid").ap()
    bq0_d = nc.inline_tensor(band_q0, "band_q0").ap()
    bq7_d = nc.inline_tensor(band_q7, "band_q7").ap()
    id_d = nc.inline_tensor(np.eye(P, dtype=ml_dtypes.bfloat16), "ident").ap()
    idh_d = nc.inline_tensor(np.eye(P, dtype=np.float16), "identh").ap()
    ones_d = nc.inline_tensor(np.ones((P, NT), dtype=ml_dtypes.bfloat16), "ones").ap()
    onesh_d = nc.inline_tensor(np.ones((P, P), dtype=np.float16), "onesh").ap()

    with tile.TileContext(nc) as tc, ExitStack() as ctx:
        pers = ctx.enter_context(tc.tile_pool(name="pers", bufs=1))
        wpool = ctx.enter_context(tc.tile_pool(name="w", bufs=1))
        spool = ctx.enter_context(tc.tile_pool(name="stat", bufs=1))
        hpool = ctx.enter_context(tc.tile_pool(name="h1", bufs=3))
        tpool = ctx.enter_context(tc.tile_pool(name="tanh", bufs=3))
        atpool = ctx.enter_context(tc.tile_pool(name="attn", bufs=4))
        smpool = ctx.enter_context(tc.tile_pool(name="small", bufs=8))
        cpool = ctx.enter_context(tc.tile_pool(name="conv", bufs=4))
        stpool = ctx.enter_context(tc.tile_pool(name="strip", bufs=2))
        lnpool = ctx.enter_context(tc.tile_pool(name="ln", bufs=2))
        psum = ctx.enter_context(tc.tile_pool(name="ps", bufs=6, space="PSUM"))

        ident = pers.tile([P, P], BF16, tag="ident")
        nc.sync.dma_start(ident, id_d)
        identh = pers.tile([P, P], FP16, tag="identh")
        nc.sync.dma_start(identh, idh_d)
        ones = pers.tile([P, NT], BF16, tag="ones")
        nc.sync.dma_start(ones, ones_d)
        onesh = pers.tile([P, P], FP16, tag="onesh")
        nc.sync.dma_start(onesh, onesh_d)
        bands = {}
        for nm, dd in (("mid", bmid_d), ("q0", bq0_d), ("q7", bq7_d)):
            bt = pers.tile([P, KW], BF16, tag=f"band_{nm}")
            nc.sync.dma_start(bt, dd)
            bands[nm] = bt

        x = pers.tile([P, DC, T], BF16, tag="x")
        with tc.tile_pool(name="xin", bufs=2) as xinp:
            for kc in range(DC):
                xf = xinp.tile([P, T], F32, name=f"xf{kc}", tag="xf")
                nc.sync.dma_start(
                    xf, xt_d.rearrange("(c p) t -> c p t", p=P)[kc])
                nc.vector.tensor_copy(out=x[:, kc], in_=xf)

        xh = pers.tile([P, DC, T], BF16, tag="xhat")
        qT = pers.tile([P, DC, T], BF16, tag="qT")
        kT = pers.tile([P, DC, T + 64], BF16, tag="kT")
        # vS[p, j, :] holds V row (j*128 - 32 + p): a partition-shifted V so
        # every AV matmul runs at base partition 0.  Slots never written
        # (keys <0 and >=T) are zeroed once here.
        vS = pers.tile([P, TB + 1, D], BF16, tag="vS")
        nc.vector.memset(vS[0:32, 0, :], 0.0)
        nc.vector.memset(vS[32:64, TB, :], 0.0)
        oT = pers.tile([P, DC, T], BF16, tag="oT")
        c2 = pers.tile([P, CC, T], FP16, tag="c2")
        c3 = pers.tile([P, CC, T], BF16, tag="c3")
        nc.vector.memset(kT[:, :, 0:32], 0.0)
        nc.vector.memset(kT[:, :, 32 + T:], 0.0)

        def rstd16(var, r):
            """r = 1/sqrt(var), bf16 [P,N] tiles, DVE 16-bit-mode ops.

            Quake seed: bitcast(MAGIC - (i>>1)) == bitcast(~(i>>1) + MAGIC+1)
            (signed int16; no overflow since var > 0), then 2 Newton steps.
            """
            vi = var.bitcast(I16)
            ri = r.bitcast(I16)
            nc.vector.tensor_scalar(
                out=ri, in0=vi, scalar1=1, scalar2=-1,
                op0=OP.logical_shift_right, op1=OP.bitwise_xor)
            nc.vector.tensor_scalar_add(out=ri, in0=ri, scalar1=MAGICBF + 1)
            t1 = spool.tile(list(var.shape), BF16, tag="nrt")
            for _ in range(2):
                nc.vector.scalar_tensor_tensor(
                    out=t1, in0=r, scalar=1.0, in1=r, op0=OP.bypass, op1=OP.mult)
                nc.vector.scalar_tensor_tensor(
                    out=t1, in0=t1, scalar=-0.5, in1=var, op0=OP.mult, op1=OP.mult)
                nc.vector.scalar_tensor_tensor(
                    out=r, in0=t1, scalar=1.5, in1=r, op0=OP.add, op1=OP.mult)

        def rstd32(var, r):
            """r = 1/sqrt(var), f32 [P,1] tiles (GroupNorm scalar path)."""
            vi = var.bitcast(I32)
            ri = r.bitcast(I32)
            nc.vector.tensor_scalar(
                out=ri, in0=vi, scalar1=1, scalar2=-1,
                op0=OP.logical_shift_right, op1=OP.bitwise_xor)
            nc.vector.tensor_scalar_add(out=ri, in0=ri, scalar1=MAGIC32 + 1)
            t1 = smpool.tile(list(var.shape), F32, tag="nrt32")
            for _ in range(3):
                nc.vector.tensor_tensor(t1, r, r, OP.mult)
                nc.vector.scalar_tensor_tensor(
                    out=t1, in0=t1, scalar=-0.5, in1=var, op0=OP.mult, op1=OP.mult)
                nc.vector.scalar_tensor_tensor(
                    out=r, in0=t1, scalar=1.5, in1=r, op0=OP.add, op1=OP.mult)

        def emit_ln(src, dst, lidx, which, out_stream=None, fin_sb=None):
            """LN over the feature (partition) axis of src -> dst, both
            [P, DC, T] bf16.  which selects the gain row pair."""
            x2 = spool.tile([P, DC, T], BF16, tag="x2")
            for kc in range(DC):
                nc.scalar.square(out=x2[:, kc], in_=src[:, kc])
            r_h = spool.tile([P, T], BF16, tag="r_h")
            mr_h = spool.tile([P, T], BF16, tag="mr_h")
            for tci in range(TC):
                sl = slice(tci * NT, (tci + 1) * NT)
                ps_s = psum.tile([P, NT], F32, tag="mm")
                ps_q = psum.tile([P, NT], F32, tag="mm")
                for kc in range(DC):
                    nc.tensor.matmul(ps_s, lhsT=ones[:, 0:P], rhs=src[:, kc, sl],
                                     start=(kc == 0), stop=(kc == DC - 1))
                for kc in range(DC):
                    nc.tensor.matmul(ps_q, lhsT=ones[:, 0:P], rhs=x2[:, kc, sl],
                                     start=(kc == 0), stop=(kc == DC - 1))
                m_t = lnpool.tile([P, NT], BF16, tag="m_t")
                q_t = lnpool.tile([P, NT], BF16, tag="q_t")
                nc.scalar.mul(out=m_t, in_=ps_s, mul=1.0 / D)
                nc.scalar.mul(out=q_t, in_=ps_q, mul=1.0 / D)
                var = lnpool.tile([P, NT], BF16, tag="var")
                nc.vector.scalar_tensor_tensor(
                    out=var, in0=m_t, scalar=1.0, in1=m_t,
                    op0=OP.bypass, op1=OP.mult)
                nc.vector.scalar_tensor_tensor(
                    out=var, in0=var, scalar=-1.0, in1=q_t,
                    op0=OP.mult, op1=OP.add)
                r16 = lnpool.tile([P, NT], BF16, tag="r16")
                rstd16(var, r16)
                nc.vector.tensor_copy(out=r_h[:, sl], in_=r16)
                nc.vector.scalar_tensor_tensor(
                    out=mr_h[:, sl], in0=m_t, scalar=1.0, in1=r16,
                    op0=OP.bypass, op1=OP.mult)
            g_sb = None
            if w_gains is not None:
                g_sb = spool.tile([P, 2, DC], F32, tag="g_sb")
                nc.sync.dma_start(
                    g_sb, w_gains[lidx, 2 * which : 2 * which + 2]
                    .rearrange("g (c p) -> p g c", p=P))
            for kc in range(DC):
                for tci in range(TC):
                    sl = slice(tci * NT, (tci + 1) * NT)
                    u = tpool.tile([P, NT], BF16, tag="ln_u")
                    nc.vector.scalar_tensor_tensor(
                        out=u, in0=src[:, kc, sl], scalar=1.0, in1=r_h[:, sl],
                        op0=OP.bypass, op1=OP.mult)
                    tgt = dst[:, kc, sl]
                    nc.vector.scalar_tensor_tensor(
                        out=tgt, in0=u, scalar=1.0, in1=mr_h[:, sl],
                        op0=OP.bypass, op1=OP.subtract)
                    if g_sb is not None:
                        nc.scalar.activation(
                            out=tgt, in_=tgt, func=AF.Identity,
                            bias=g_sb[:, 1, kc : kc + 1], scale=g_sb[:, 0, kc : kc + 1])
                    if out_stream is not None:
                        pool_o, dview = out_stream
                        of = pool_o.tile([P, NT], F32, tag="of")
                        nc.vector.scalar_tensor_tensor(
                            out=of, in0=u, scalar=1.0, in1=mr_h[:, sl],
                            op0=OP.bypass, op1=OP.subtract)
                        if g_sb is not None:
                            nc.scalar.activation(
                                out=of, in_=of, func=AF.Identity,
                                bias=g_sb[:, 1, kc : kc + 1],
                                scale=g_sb[:, 0, kc : kc + 1])
                        if fin_sb is not None:
                            nc.scalar.activation(
                                out=of, in_=of, func=AF.Identity,
                                bias=fin_sb[:, 1, kc : kc + 1],
                                scale=fin_sb[:, 0, kc : kc + 1])
                        nc.sync.dma_start(dview[:, kc, sl], of)

        def load_w(dram, lidx, tag):
            _, fin, fout = dram.shape
            wt = wpool.tile([P, fin // P, fout], BF16, tag=tag)
            nc.sync.dma_start(wt, dram[lidx].rearrange("(c p) f -> p c f", p=P))
            return wt

        bias_sb = [None]

        def bias_mm(ps, row, mslice, tcslice_n):
            """Add bias row (features mslice) into psum via a K=1 matmul."""
            if bias_sb[0] is None:
                return
            nc.tensor.matmul(
                ps, lhsT=bias_sb[0][0:1, row, mslice], rhs=ones[0:1, 0:tcslice_n],
                start=False, stop=True, skip_group_check=True)

        def emit_ffn(wa_d, wb_d, rows, lidx, src):
            w1 = load_w(wa_d, lidx, "w1")
            w2 = load_w(wb_d, lidx, "w2")
            for tci in range(TC):
                sl = slice(tci * NT, (tci + 1) * NT)
                acc = [psum.tile([P, NT], F32, tag="mm", name=f"acc{i}")
                       for i in range(DC)]
                for m in range(FC):
                    ph = psum.tile([P, NT], F32, tag="mm")
                    for kc in range(DC):
                        nc.tensor.matmul(
                            ph, lhsT=w1[:, kc, m * P : (m + 1) * P], rhs=src[:, kc, sl],
                            start=(kc == 0), stop=(kc == DC - 1 and not bias_gen))
                    bias_mm(ph, rows[0], slice(m * P, (m + 1) * P), NT)
                    hb = hpool.tile([P, NT], BF16, tag="h1")
                    nc.scalar.activation(out=hb, in_=ph, func=AF.Silu)
                    for dcc in range(DC):
                        nc.tensor.matmul(
                            acc[dcc], lhsT=w2[:, m, dcc * P : (dcc + 1) * P], rhs=hb,
                            start=(m == 0), stop=(m == FC - 1 and not bias_gen),
                            skip_group_check=True)
                for dcc in range(DC):
                    bias_mm(acc[dcc], rows[1], slice(dcc * P, (dcc + 1) * P), NT)
                    nc.vector.scalar_tensor_tensor(
                        out=x[:, dcc, sl], in0=acc[dcc], scalar=1.0,
                        in1=x[:, dcc, sl], op0=OP.bypass, op1=OP.add)

        for l in range(LAYERS):
            if bias_gen:
                bt = wpool.tile([1, 10, 2 * EC], BF16, tag="bias")
                nc.sync.dma_start(bt, w_bias[l])
                bias_sb[0] = bt

            # ===== FFN1 (half residual) =====
            if "f" in PHASES:
                if l == 0 or ln_gen:
                    emit_ln(x, xh, l, 0)
                    src1 = xh
                else:
                    src1 = x  # already unit-normalized by previous blk LN
                emit_ffn(w_f1a, w_f1b, (BR_F1B1, BR_F1B2), l, src1)

            # ===== local windowed MHSA =====
            if "a" in PHASES or "A" in PHASES:
                emit_ln(x, xh, l, 1)
                wq = load_w(w_q, l, "wq")
                wk = load_w(w_kk, l, "wk")
                wv = load_w(w_v, l, "wv")
                wo = load_w(w_o, l, "wo")
                for m in range(DC):
                    for tci in range(TC):
                        sl = slice(tci * NT, (tci + 1) * NT)
                        pq = psum.tile([P, NT], F32, tag="mm")
                        for kc in range(DC):
                            nc.tensor.matmul(
                                pq, lhsT=wq[:, kc, m * P : (m + 1) * P],
                                rhs=xh[:, kc, sl],
                                start=(kc == 0), stop=(kc == DC - 1 and not bias_gen))
                        bias_mm(pq, BR_Q, slice(m * P, (m + 1) * P), NT)
                        nc.scalar.copy(out=qT[:, m, sl], in_=pq)
                        pk = psum.tile([P, NT], F32, tag="mm")
                        for kc in range(DC):
                            nc.tensor.matmul(
                                pk, lhsT=wk[:, kc, m * P : (m + 1) * P],
                                rhs=xh[:, kc, sl],
                                start=(kc == 0), stop=(kc == DC - 1 and not bias_gen))
                        bias_mm(pk, BR_K, slice(m * P, (m + 1) * P), NT)
                        nc.scalar.copy(
                            out=kT[:, m, 32 + tci * NT : 32 + (tci + 1) * NT], in_=pk)
                for tb in range(TB):
                    pv = psum.tile([P, NT], F32, tag="mm")
                    for kc in range(DC):
                        nc.tensor.matmul(
                            pv, lhsT=xh[:, kc, tb * P : (tb + 1) * P],
                            rhs=wv[:, kc, 0:D],
                            start=(kc == 0), stop=(kc == DC - 1 and not bias_gen))
                    if bias_gen:
                        nc.tensor.matmul(
                            pv, lhsT=ones[0:1, 0:P], rhs=bias_sb[0][0:1, BR_V, 0:D],
                            start=False, stop=True, skip_group_check=True)
                    vstage = hpool.tile([P, D], BF16, tag="vstage")
                    nc.vector.tensor_copy(out=vstage, in_=pv)
                    nc.sync.dma_start(vS[32:128, tb, :], vstage[0:96, :])
                    nc.sync.dma_start(vS[0:32, tb + 1, :], vstage[96:128, :])
                for hp in range(DC):
                    for qb in range(TB):
                        band = bands["q0"] if qb == 0 else (
                            bands["q7"] if qb == TB - 1 else bands["mid"])
                        po = psum.tile([P, P], F32, tag="mm")
                        for hh in range(2):
                            pr = slice(hh * DH, (hh + 1) * DH)
                            ps_s = psum.tile([P, KW], F32, tag="mm")
                            nc.tensor.matmul(
                                ps_s, lhsT=qT[pr, hp, qb * P : (qb + 1) * P],
                                rhs=kT[pr, hp, qb * P : qb * P + KW],
                                start=True, stop=True)
                            at = atpool.tile([P, KW], BF16, tag="at")
                            nc.scalar.activation(out=at, in_=ps_s, func=AF.Exp)
                            lsum = smpool.tile([P, 1], F32, tag="l")
                            nc.vector.scalar_tensor_tensor(
                                out=at, in0=at, scalar=1.0, in1=band,
                                op0=OP.bypass, op1=OP.mult, accum_out=lsum)
                            rl = smpool.tile([P, 1], F32, tag="rl")
                            nc.vector.reciprocal(out=rl, in_=lsum)
                            nc.vector.tensor_scalar_mul(out=at, in0=at, scalar1=rl)
                            pt = psum.tile([P, KW], BF16, tag="mm")
                            nc.tensor.transpose(pt[:, 0:128], at[:, 0:128], ident)
                            nc.tensor.transpose(pt[0:64, 128:192],
                                                at[64:128, 128:192],
                                                ident[64:128, 64:128])
                            asb = atpool.tile([P, KW], BF16, tag="asb")
                            if hh == 0:
                                nc.vector.tensor_copy(out=asb, in_=pt)
                            else:
                                nc.scalar.copy(out=asb, in_=pt)
                            h = hp * 2 + hh
                            hc = slice(h * DH, (h + 1) * DH)
                            nc.tensor.matmul(
                                po[pr, :], lhsT=vS[:, qb, hc], rhs=asb[:, 0:128],
                                start=True, stop=False, skip_group_check=True)
                            nc.tensor.matmul(
                                po[pr, 64:128], lhsT=vS[0:64, qb + 1, hc],
                                rhs=asb[0:64, 128:192],
                                start=False, stop=True, skip_group_check=True)
                        nc.vector.tensor_copy(
                            out=oT[:, hp, qb * P : (qb + 1) * P], in_=po)
                if "A" not in PHASES:
                    for tci in range(TC):
                        sl = slice(tci * NT, (tci + 1) * NT)
                        for m in range(DC):
                            pp = psum.tile([P, NT], F32, tag="mm")
                            for kc in range(DC):
                                nc.tensor.matmul(
                                    pp, lhsT=wo[:, kc, m * P : (m + 1) * P],
                                    rhs=oT[:, kc, sl],
                                    start=(kc == 0), stop=(kc == DC - 1 and not bias_gen))
                            bias_mm(pp, BR_O, slice(m * P, (m + 1) * P), NT)
                            nc.vector.scalar_tensor_tensor(
                                out=x[:, m, sl], in0=pp, scalar=1.0, in1=x[:, m, sl],
                                op0=OP.bypass, op1=OP.add)

            # ===== convolution module =====
            if "c" in PHASES:
                emit_ln(x, xh, l, 2)
                p1 = load_w(w_p1, l, "w1")
                p2 = load_w(w_p2, l, "w2")
                dwt = wpool.tile([P, CC, KK], F32, tag="dw")
                nc.sync.dma_start(dwt, w_dw[l])
                dwb_sb = None
                if dwb_gen:
                    dwb_sb = wpool.tile([P, CC], F32, tag="dwb")
                    nc.sync.dma_start(dwb_sb, w_dwb[l])
                for mp in range(0, CC, 2):
                    pair = (mp, mp + 1)
                    cps = {}
                    strips = {}
                    for m in pair:
                        # diag strips for the 15 odd PE taps, DMA'd from the
                        # host-precomputed tensor
                        st = stpool.tile([P, len(PE_TAPS), P], FP16, tag="strip")
                        nc.sync.dma_start(st, w_strip[l, m])
                        strips[m] = st
                        cp = cpool.tile([P, KK - 1 + T], FP16, tag="cp")
                        nc.vector.memset(cp[:, 0 : KK // 2], 0.0)
                        nc.vector.memset(cp[:, KK // 2 + T :], 0.0)
                        cps[m] = cp
                        for tci in range(TC):
                            sl = slice(tci * NT, (tci + 1) * NT)
                            pb = psum.tile([P, NT], F32, tag="mm")
                            for kc in range(DC):
                                nc.tensor.matmul(
                                    pb, lhsT=p1[:, kc, EC + m * P : EC + (m + 1) * P],
                                    rhs=xh[:, kc, sl],
                                    start=(kc == 0), stop=(kc == DC - 1 and not bias_gen))
                            bias_mm(pb, BR_P1, slice(EC + m * P, EC + (m + 1) * P), NT)
                            tb_ = tpool.tile([P, NT], FP16, tag="th")
                            nc.scalar.activation(out=tb_, in_=pb, func=AF.Tanh, scale=0.5)
                            pa = psum.tile([P, NT], F32, tag="mm")
                            for kc in range(DC):
                                nc.tensor.matmul(
                                    pa, lhsT=p1[:, kc, m * P : (m + 1) * P],
                                    rhs=xh[:, kc, sl],
                                    start=(kc == 0), stop=(kc == DC - 1 and not bias_gen))
                            bias_mm(pa, BR_P1, slice(m * P, (m + 1) * P), NT)
                            nc.vector.scalar_tensor_tensor(
                                out=cp[:, KK // 2 + tci * NT : KK // 2 + (tci + 1) * NT],
                                in0=tb_, scalar=1.0, in1=pa, op0=OP.add, op1=OP.mult)
                    # PE: odd taps via strip matmuls, f32 psum accumulation
                    pcs = {}
                    for m in pair:
                        for tci in range(TC):
                            pc = psum.tile([P, NT], F32, tag="mm",
                                           name=f"pc{m}_{tci}")
                            for j, kk in enumerate(PE_TAPS):
                                nc.tensor.matmul(
                                    pc, lhsT=strips[m][:, j, :],
                                    rhs=cps[m][:, kk + tci * NT : kk + tci * NT + NT],
                                    start=(j == 0), stop=(j == len(PE_TAPS) - 1),
                                    skip_group_check=True)
                            pcs[(m, tci)] = pc
                    # DVE: even taps as fp16 acc chains (4 interleaved chains)
                    for j, kk in enumerate(DVE_TAPS):
                        for m in pair:
                            for tci in range(TC):
                                csl = c2[:, m, tci * NT : (tci + 1) * NT]
                                src_ = cps[m][:, kk + tci * NT : kk + tci * NT + NT]
                                if j == 0:
                                    nc.vector.tensor_scalar_mul(
                                        out=csl, in0=src_,
                                        scalar1=dwt[:, m, kk : kk + 1])
                                else:
                                    nc.vector.scalar_tensor_tensor(
                                        out=csl, in0=src_,
                                        scalar=dwt[:, m, kk : kk + 1], in1=csl,
                                        op0=OP.mult, op1=OP.add)
                    # combine PE + DVE halves (+ optional dw bias)
                    for m in pair:
                        for tci in range(TC):
                            csl = c2[:, m, tci * NT : (tci + 1) * NT]
                            dwb_s = dwb_sb[:, m : m + 1] if dwb_sb is not None else 0.0
                            nc.vector.scalar_tensor_tensor(
                                out=csl, in0=pcs[(m, tci)], scalar=dwb_s,
                                in1=csl, op0=OP.add, op1=OP.add)
                # GroupNorm(1 group over [EC, T]) stats
                parts = []
                for tci in range(TC):
                    sl = slice(tci * NT, (tci + 1) * NT)
                    ps_s = psum.tile([P, NT], F32, tag="mm")
                    ps_q = psum.tile([P, NT], F32, tag="mm")
                    for m in range(CC):
                        nc.tensor.matmul(ps_s, lhsT=onesh[:, 0:P], rhs=c2[:, m, sl],
                                         start=(m == 0), stop=(m == CC - 1))
                    for m in range(CC):
                        cs = hpool.tile([P, NT], FP16, tag="cs")
                        nc.scalar.square(out=cs, in_=c2[:, m, sl])
                        nc.tensor.matmul(ps_q, lhsT=onesh[:, 0:P], rhs=cs,
                                         start=(m == 0), stop=(m == CC - 1))
                    rs = smpool.tile([P, 1], F32, tag=f"gs{tci}")
                    rq = smpool.tile([P, 1], F32, tag=f"gq{tci}")
                    nc.vector.tensor_reduce(out=rs, in_=ps_s,
                                            axis=mybir.AxisListType.X, op=OP.add)
                    nc.vector.tensor_reduce(out=rq, in_=ps_q,
                                            axis=mybir.AxisListType.X, op=OP.add)
                    parts.append((rs, rq))
                gs = smpool.tile([P, 1], F32, tag="gsum")
                gq = smpool.tile([P, 1], F32, tag="gqsum")
                nc.vector.tensor_tensor(gs, parts[0][0], parts[1][0], OP.add)
                nc.vector.tensor_tensor(gq, parts[0][1], parts[1][1], OP.add)
                mg = smpool.tile([P, 1], F32, tag="mg")
                nc.vector.tensor_scalar_mul(out=mg, in0=gs, scalar1=1.0 / (EC * T))
                msqg = smpool.tile([P, 1], F32, tag="msqg")
                nc.vector.tensor_tensor(msqg, mg, mg, OP.mult)
                varg = smpool.tile([P, 1], F32, tag="varg")
                nc.vector.scalar_tensor_tensor(
                    out=varg, in0=gq, scalar=1.0 / (EC * T), in1=msqg,
                    op0=OP.mult, op1=OP.subtract)
                nc.vector.tensor_scalar_add(out=varg, in0=varg, scalar1=EPS)
                rg = smpool.tile([P, 1], F32, tag="rg")
                rstd32(varg, rg)
                # A = gn_g * r ; B = gn_b - m * A    (per-channel, [P, CC])
                gaff = spool.tile([P, 2, CC], F32, tag="gaff")
                nc.sync.dma_start(gaff, w_gn[l].rearrange("g (c p) -> p g c", p=P))
                a_t = spool.tile([P, CC], F32, tag="a_t")
                nc.vector.tensor_scalar_mul(out=a_t, in0=gaff[:, 0], scalar1=rg)
                mneg = smpool.tile([P, 1], F32, tag="mneg")
                nc.vector.tensor_scalar_mul(out=mneg, in0=mg, scalar1=-1.0)
                b_t = spool.tile([P, CC], F32, tag="b_t")
                nc.vector.scalar_tensor_tensor(
                    out=b_t, in0=a_t, scalar=mneg, in1=gaff[:, 1],
                    op0=OP.mult, op1=OP.add)
                for m in range(CC):
                    nc.scalar.activation(
                        out=c3[:, m], in_=c2[:, m], func=AF.Silu,
                        bias=b_t[:, m : m + 1], scale=a_t[:, m : m + 1])
                for tci in range(TC):
                    sl = slice(tci * NT, (tci + 1) * NT)
                    for dcc in range(DC):
                        pp = psum.tile([P, NT], F32, tag="mm")
                        for m in range(CC):
                            nc.tensor.matmul(
                                pp, lhsT=p2[:, m, dcc * P : (dcc + 1) * P],
                                rhs=c3[:, m, sl],
                                start=(m == 0), stop=(m == CC - 1 and not bias_gen))
                        bias_mm(pp, BR_P2, slice(dcc * P, (dcc + 1) * P), NT)
                        nc.vector.scalar_tensor_tensor(
                            out=x[:, dcc, sl], in0=pp, scalar=1.0, in1=x[:, dcc, sl],
                            op0=OP.bypass, op1=OP.add)

            # ===== FFN2 (half residual) =====
            if "2" in PHASES:
                emit_ln(x, xh, l, 3)
                emit_ffn(w_f2a, w_f2b, (BR_F2B1, BR_F2B2), l, xh)

            # ===== per-block LN =====
            if "b" in PHASES:
                if l == LAYERS - 1:
                    with tc.tile_pool(name="outp", bufs=3) as op_:
                        fin_sb = None
                        if w_fin is not None:
                            fin_sb = spool.tile([P, 2, DC], F32, tag="fin_sb")
                            nc.sync.dma_start(
                                fin_sb, w_fin.rearrange("g (c p) -> p g c", p=P))
                        emit_ln(x, x, l, 4,
                                out_stream=(op_, out_d.rearrange(
                                    "(c p) t -> p c t", p=P)),
                                fin_sb=fin_sb)
                else:
                    emit_ln(x, x, l, 4)

        if "b" not in PHASES or LAYERS == 0:
            # debug path: dump current x (or oT for 'A') as output
            with tc.tile_pool(name="outp", bufs=3) as op_:
                srcd = oT if "A" in PHASES else x
                dview = out_d.rearrange("(c p) t -> p c t", p=P)
                for kc in range(DC):
                    for tci in range(TC):
                        sl = slice(tci * NT, (tci + 1) * NT)
                        of = op_.tile([P, NT], F32, tag="of")
                        nc.vector.tensor_copy(out=of, in_=srcd[:, kc, sl])
                        nc.sync.dma_start(dview[:, kc, sl], of)

    nc.finalize()
    return nc


_PROG_CACHE = {}


def _get_program(flags):
    key = tuple(sorted(flags.items())) + (LAYERS, PHASES)
    if key not in _PROG_CACHE:
        _PROG_CACHE[key] = build_program(flags)
    return _PROG_CACHE[key]


def kernel(**inputs):
    global LAST_RESULT
    f32 = lambda a: np.asarray(a, dtype=np.float32)
    bf = lambda a: np.ascontiguousarray(f32(a).astype(ml_dtypes.bfloat16))
    x = f32(inputs["x"])                       # [B, T, D]

    def triv(names_vals):
        return all(bool(np.all(f32(inputs[n]) == v)) for n, v in names_vals)

    ln_trivial = triv(
        [(f"{p}_ln_g", 1.0) for p in ("ffn1", "attn", "conv", "ffn2", "blk")]
        + [(f"{p}_ln_b", 0.0) for p in ("ffn1", "attn", "conv", "ffn2", "blk")])
    final_trivial = triv([("final_ln_g", 1.0), ("final_ln_b", 0.0)])
    bias_trivial = triv([(n, 0.0) for n in (
        "ffn1_b1", "ffn1_b2", "qkv_b", "outp_b", "pw1_b", "pw2_b",
        "ffn2_b1", "ffn2_b2")])
    dwb_trivial = triv([("dw_b", 0.0)])
    flags = dict(ln_trivial=ln_trivial, final_trivial=final_trivial,
                 bias_trivial=bias_trivial, dwb_trivial=dwb_trivial)

    nc = _get_program(flags)

    qkv = f32(inputs["qkv_w"])                # [L, D, 3D]
    dwp = (f32(inputs["dw_w"]).reshape(L, EC, KK) * 0.5).reshape(L, CC, P, KK)
    dw = dwp.transpose(0, 2, 1, 3)            # [L, P, CC, K]
    npe = len(PE_TAPS)
    strips = np.zeros((L, CC, P, npe, P), dtype=np.float16)
    idx = np.arange(P)
    # strips[l, c, p, j, p] = dwp[l, c, p, PE_TAPS[j]]
    strips[:, :, idx, :, idx] = dwp[:, :, :, list(PE_TAPS)].transpose(
        2, 0, 1, 3).astype(np.float16)
    gn_aff = np.stack([f32(inputs["gn_g"]), f32(inputs["gn_b"])], axis=1)

    common = {
        "f1w1": bf(inputs["ffn1_w1"]),
        "f1w2": bf(f32(inputs["ffn1_w2"]) * 0.5),
        "f2w1": bf(inputs["ffn2_w1"]),
        "f2w2": bf(f32(inputs["ffn2_w2"]) * 0.5),
        "wq": bf(qkv[:, :, 0:D] * (DH ** -0.5)),
        "wk": bf(qkv[:, :, D : 2 * D]),
        "wv": bf(qkv[:, :, 2 * D : 3 * D]),
        "wo": bf(inputs["outp_w"]),
        "pw1": bf(inputs["pw1_w"]),
        "pw2": bf(inputs["pw2_w"]),
        "dw": np.ascontiguousarray(dw.astype(np.float32)),
        "strips": np.ascontiguousarray(strips),
        "gn_aff": np.ascontiguousarray(gn_aff.astype(np.float32)),
    }
    if not ln_trivial:
        rows = []
        for pfx in ("ffn1", "attn", "conv", "ffn2", "blk"):
            rows.append(f32(inputs[f"{pfx}_ln_g"]))
            rows.append(f32(inputs[f"{pfx}_ln_b"]))
        common["ln_gains"] = np.ascontiguousarray(
            np.stack(rows, axis=1).astype(np.float32))  # [L, 10, D]
    if not final_trivial:
        common["final_aff"] = np.ascontiguousarray(np.stack(
            [f32(inputs["final_ln_g"]), f32(inputs["final_ln_b"])]).astype(np.float32))
    if not bias_trivial:
        bias = np.zeros((L, 10, 2 * EC), np.float32)
        qb = f32(inputs["qkv_b"])
        bias[:, BR_F1B1, :FF] = f32(inputs["ffn1_b1"])
        bias[:, BR_F1B2, :D] = f32(inputs["ffn1_b2"]) * 0.5
        bias[:, BR_Q, :D] = qb[:, 0:D] * (DH ** -0.5)
        bias[:, BR_K, :D] = qb[:, D : 2 * D]
        bias[:, BR_V, :D] = qb[:, 2 * D : 3 * D]
        bias[:, BR_O, :D] = f32(inputs["outp_b"])
        bias[:, BR_P1, : 2 * EC] = f32(inputs["pw1_b"])
        bias[:, BR_P2, :D] = f32(inputs["pw2_b"])
        bias[:, BR_F2B1, :FF] = f32(inputs["ffn2_b1"])
        bias[:, BR_F2B2, :D] = f32(inputs["ffn2_b2"]) * 0.5
        common["biases"] = bf(bias)
    if not dwb_trivial:
        dwb = f32(inputs["dw_b"]).reshape(L, CC, P).transpose(0, 2, 1)
        common["dwb"] = np.ascontiguousarray(dwb.astype(np.float32))

    in_maps = []
    for c in range(B):
        m = dict(common)
        m["x_t"] = np.ascontiguousarray(x[c].T)   # [D, T] fp32
        in_maps.append(m)

    res = run_bass_kernel_spmd(
        nc, in_maps, core_ids=list(range(B)), trace=TRACE, **TRACE_KW)
    LAST_RESULT = res
    out = np.stack([r["out_t"].T for r in res.results]).astype(np.float32)
    return out


if __name__ == "__main__":
    rng = np.random.default_rng(0)
    ins = {"x": rng.standard_normal((B, T, D), dtype=np.float32)}
    # minimal smoke test requires full inputs; use test.py instead
    print("use test.py")


# revision 20
# speedup vs baseline: 1.1059x; 1.1059x over previous
"""ConformerDecoder Trainium2 Bass kernel (v2).

Sharding: pure data-parallel over batch B=8 -> one sample per NeuronCore.

Activations live in "transposed" layout [feature-on-partitions, T-free] so the
whole matmul chain runs with weights as natural lhsT operands, bf16 matmuls
with fp32 PSUM accumulation.

v2 changes vs baseline (3.37ms -> target ~1.3ms):
- Attention windows shrunk 384 -> 192 key-columns (W=64 band fits in a
  32-shifted 192 window): halves scores/exp/mask/AV work on every engine.
  AV uses partition-offset sub-matmuls against the natural V tiles, so no
  shifted V copy is needed; edge blocks simply skip the out-of-range pieces.
- Depthwise conv K=31 split across engines: 16 even taps as fp16
  scalar_tensor_tensor acc-chains on DVE (4x perf mode: 16-bit, SBUF-only,
  4B-aligned via even shifts), 15 odd taps on PE as diag-strip matmuls
  (strips built by the ACT engine, not DVE), combined during the PSUM
  evacuation.  No more 31x128-wasteful all-PE conv and no DVE strip stall.
- LayerNorm rstd via the fp16 Quake bit-trick (int16 shift/xor seed + 2
  inline Newton steps, all 4x-mode DVE ops) instead of slow f32 Newton
  chains; mean/sumsq scaled copies moved to ACT.  GroupNorm rstd same trick
  in f32 [P,1] (replaces a 14-iteration Newton chain that stalled PE).
- GroupNorm affine+SiLU fused into a single per-chunk ACT op (scale/bias are
  per-partition APs).
- Engine rebalance: psum evacuations split between ACT and DVE.
"""

import os
import sys
from contextlib import ExitStack

for _p in ("/opt/trn_rl_repo",):
    if _p not in sys.path:
        sys.path.insert(0, _p)

import numpy as np
import ml_dtypes

import concourse.bass as bass
import concourse.tile as tile
from concourse import bacc
from concourse import mybir
from concourse.bass_utils import run_bass_kernel_spmd

BF16 = mybir.dt.bfloat16
FP16 = mybir.dt.float16
I16 = mybir.dt.int16
I32 = mybir.dt.int32
F32 = mybir.dt.float32
AF = mybir.ActivationFunctionType
OP = mybir.AluOpType

L, D, H, T, B = 4, 512, 8, 1024, 8
FF = 4 * D            # 2048
EC = 2 * D            # 1024 conv channels
KK = 31               # conv kernel size
WIN = 64              # attention window
DH = D // H           # 64
P = 128
DC = D // P           # 4 feature chunks
FC = FF // P          # 16
CC = EC // P          # 8
TB = T // P           # 8 token blocks
NT = 512              # matmul moving free dim
TC = T // NT          # 2 t-columns
KW = 192              # attention window width (32-shifted)
EPS = 1e-5

# bf16 / fp32 rsqrt bit-trick magics
MAGICBF = 0x5F34
MAGIC32 = 0x5F3759DF

TRACE = False          # set by test.py for profiling runs
TRACE_KW = {}
LAST_RESULT = None     # BassKernelResults of last run (read by test.py)
LAYERS = int(os.environ.get("CONF_LAYERS", str(L)))
PHASES = os.environ.get("CONF_PHASES", "fac2b")

# bias row indices in the packed bias tensor
BR_F1B1, BR_F1B2, BR_Q, BR_K, BR_V, BR_O, BR_P1, BR_P2, BR_F2B1, BR_F2B2 = range(10)

# Per-partition-scalar DVE ops run at 1x on TRN2, so DVE taps are ~3x the
# cost of a PE strip-tap; keep only a few on DVE to overlap the PE stream.
DVE_TAPS = tuple(range(0, KK, 8))   # 4 taps: 0, 8, 16, 24
PE_TAPS = tuple(k for k in range(KK) if k % 8 != 0)   # 27 strip-matmul taps


def _band_masks():
    """[128, KW] multiplicative masks (mid, qb0, qb7) in bf16.

    Window column c for q-block qb is absolute key k = qb*128 - 32 + c;
    row i is query q = qb*128 + i.  Valid iff |q-k| <= WIN/2 and 0<=k<T,
    i.e. c in [i, i+64], plus c>=32 for qb=0 and c<=159 for qb=7.
    """
    i = np.arange(P)[:, None]
    c = np.arange(KW)[None, :]
    band = np.abs(i + 32 - c) <= WIN // 2
    q0 = band & (c >= 32)
    q7 = band & (c <= 159)
    to = lambda m: np.ascontiguousarray(m.astype(ml_dtypes.bfloat16))
    return to(band), to(q0), to(q7)


def build_program(flags):
    ln_gen = not flags["ln_trivial"]
    bias_gen = not flags["bias_trivial"]
    fin_gen = not flags["final_trivial"]
    dwb_gen = not flags["dwb_trivial"]

    nc = bacc.Bacc("TRN2", target_bir_lowering=False, debug=False)

    xt_d = nc.dram_tensor("x_t", [D, T], F32, kind="ExternalInput").ap()
    out_d = nc.dram_tensor("out_t", [D, T], F32, kind="ExternalOutput").ap()

    def win(name, shape):
        return nc.dram_tensor(name, shape, BF16, kind="ExternalInput").ap()

    w_f1a = win("f1w1", [L, D, FF])
    w_f1b = win("f1w2", [L, FF, D])
    w_f2a = win("f2w1", [L, D, FF])
    w_f2b = win("f2w2", [L, FF, D])
    w_q = win("wq", [L, D, D])
    w_kk = win("wk", [L, D, D])
    w_v = win("wv", [L, D, D])
    w_o = win("wo", [L, D, D])
    w_p1 = win("pw1", [L, D, 2 * EC])
    w_p2 = win("pw2", [L, EC, D])
    w_dw = nc.dram_tensor("dw", [L, P, CC, KK], F32, kind="ExternalInput").ap()
    w_strip = nc.dram_tensor("strips", [L, CC, P, len(PE_TAPS), P], FP16,
                             kind="ExternalInput").ap()
    w_gn = nc.dram_tensor("gn_aff", [L, 2, EC], F32, kind="ExternalInput").ap()
    w_gains = (nc.dram_tensor("ln_gains", [L, 10, D], F32, kind="ExternalInput").ap()
                if ln_gen else None)
    w_fin = (nc.dram_tensor("final_aff", [2, D], F32, kind="ExternalInput").ap()
             if fin_gen else None)
    w_bias = win("biases", [L, 10, 2 * EC]) if bias_gen else None
    w_dwb = (nc.dram_tensor("dwb", [L, P, CC], F32, kind="ExternalInput").ap()
             if dwb_gen else None)

    band_mid, band_q0, band_q7 = _band_masks()
    bmid_d = nc.inline_tensor(band_mid, "band_mid").ap()
    bq0_d = nc.inline_tensor(band_q0, "band_q0").ap()
    bq7_d = nc.inline_tensor(band_q7, "band_q7").ap()
    id_d = nc.inline_tensor(np.eye(P, dtype=ml_dtypes.bfloat16), "ident").ap()
    idh_d = nc.inline_tensor(np.eye(P, dtype=np.float16), "identh").ap()
    ones_d = nc.inline_tensor(np.ones((P, NT), dtype=ml_dtypes.bfloat16), "ones").ap()
    onesh_d = nc.inline_tensor(np.ones((P, P), dtype=np.float16), "onesh").ap()

    with tile.TileContext(nc) as tc, ExitStack() as ctx:
        pers = ctx.enter_context(tc.tile_pool(name="pers", bufs=1))
        wpool = ctx.enter_context(tc.tile_pool(name="w", bufs=1))
        spool = ctx.enter_context(tc.tile_pool(name="stat", bufs=1))
        hpool = ctx.enter_context(tc.tile_pool(name="h1", bufs=3))
        tpool = ctx.enter_context(tc.tile_pool(name="tanh", bufs=3))
        atpool = ctx.enter_context(tc.tile_pool(name="attn", bufs=4))
        smpool = ctx.enter_context(tc.tile_pool(name="small", bufs=8))
        cpool = ctx.enter_context(tc.tile_pool(name="conv", bufs=4))
        stpool = ctx.enter_context(tc.tile_pool(name="strip", bufs=2))
        lnpool = ctx.enter_context(tc.tile_pool(name="ln", bufs=2))
        psum = ctx.enter_context(tc.tile_pool(name="ps", bufs=6, space="PSUM"))

        ident = pers.tile([P, P], BF16, tag="ident")
        nc.sync.dma_start(ident, id_d)
        identh = pers.tile([P, P], FP16, tag="identh")
        nc.sync.dma_start(identh, idh_d)
        ones = pers.tile([P, NT], BF16, tag="ones")
        nc.sync.dma_start(ones, ones_d)
        onesh = pers.tile([P, P], FP16, tag="onesh")
        nc.sync.dma_start(onesh, onesh_d)
        bands = {}
        for nm, dd in (("mid", bmid_d), ("q0", bq0_d), ("q7", bq7_d)):
            bt = pers.tile([P, KW], BF16, tag=f"band_{nm}")
            nc.sync.dma_start(bt, dd)
            bands[nm] = bt

        x = pers.tile([P, DC, T], BF16, tag="x")
        with tc.tile_pool(name="xin", bufs=2) as xinp:
            for kc in range(DC):
                xf = xinp.tile([P, T], F32, name=f"xf{kc}", tag="xf")
                nc.sync.dma_start(
                    xf, xt_d.rearrange("(c p) t -> c p t", p=P)[kc])
                nc.vector.tensor_copy(out=x[:, kc], in_=xf)

        xh = pers.tile([P, DC, T], BF16, tag="xhat")
        qT = pers.tile([P, DC, T], BF16, tag="qT")
        kT = pers.tile([P, DC, T + 64], BF16, tag="kT")
        # vS[p, j, :] holds V row (j*128 - 32 + p): a partition-shifted V so
        # every AV matmul runs at base partition 0.  Slots never written
        # (keys <0 and >=T) are zeroed once here.
        vS = pers.tile([P, TB + 1, D], BF16, tag="vS")
        nc.vector.memset(vS[0:32, 0, :], 0.0)
        nc.vector.memset(vS[32:64, TB, :], 0.0)
        oT = pers.tile([P, DC, T], BF16, tag="oT")
        c2 = pers.tile([P, CC, T], FP16, tag="c2")
        c3 = pers.tile([P, CC, T], BF16, tag="c3")
        nc.vector.memset(kT[:, :, 0:32], 0.0)
        nc.vector.memset(kT[:, :, 32 + T:], 0.0)

        def rstd16(var, r):
            """r = 1/sqrt(var), bf16 [P,N] tiles, DVE 16-bit-mode ops.

            Quake seed: bitcast(MAGIC - (i>>1)) == bitcast(~(i>>1) + MAGIC+1)
            (signed int16; no overflow since var > 0), then 2 Newton steps.
            """
            vi = var.bitcast(I16)
            ri = r.bitcast(I16)
            nc.vector.tensor_scalar(
                out=ri, in0=vi, scalar1=1, scalar2=-1,
                op0=OP.logical_shift_right, op1=OP.bitwise_xor)
            nc.vector.tensor_scalar_add(out=ri, in0=ri, scalar1=MAGICBF + 1)
            t1 = spool.tile(list(var.shape), BF16, tag="nrt")
            for _ in range(2):
                nc.vector.scalar_tensor_tensor(
                    out=t1, in0=r, scalar=1.0, in1=r, op0=OP.bypass, op1=OP.mult)
                nc.vector.scalar_tensor_tensor(
                    out=t1, in0=t1, scalar=-0.5, in1=var, op0=OP.mult, op1=OP.mult)
                nc.vector.scalar_tensor_tensor(
                    out=r, in0=t1, scalar=1.5, in1=r, op0=OP.add, op1=OP.mult)

        def rstd32(var, r):
            """r = 1/sqrt(var), f32 [P,1] tiles (GroupNorm scalar path)."""
            vi = var.bitcast(I32)
            ri = r.bitcast(I32)
            nc.vector.tensor_scalar(
                out=ri, in0=vi, scalar1=1, scalar2=-1,
                op0=OP.logical_shift_right, op1=OP.bitwise_xor)
            nc.vector.tensor_scalar_add(out=ri, in0=ri, scalar1=MAGIC32 + 1)
            t1 = smpool.tile(list(var.shape), F32, tag="nrt32")
            for _ in range(3):
                nc.vector.tensor_tensor(t1, r, r, OP.mult)
                nc.vector.scalar_tensor_tensor(
                    out=t1, in0=t1, scalar=-0.5, in1=var, op0=OP.mult, op1=OP.mult)
                nc.vector.scalar_tensor_tensor(
                    out=r, in0=t1, scalar=1.5, in1=r, op0=OP.add, op1=OP.mult)

        def emit_ln(src, dst, lidx, which, out_stream=None, fin_sb=None):
            """LN over the feature (partition) axis of src -> dst, both
            [P, DC, T] bf16.  which selects the gain row pair."""
            x2 = spool.tile([P, DC, T], BF16, tag="x2")
            for kc in range(DC):
                nc.scalar.square(out=x2[:, kc], in_=src[:, kc])
            r_h = spool.tile([P, T], BF16, tag="r_h")
            mr_h = spool.tile([P, T], BF16, tag="mr_h")
            for tci in range(TC):
                sl = slice(tci * NT, (tci + 1) * NT)
                ps_s = psum.tile([P, NT], F32, tag="mm")
                ps_q = psum.tile([P, NT], F32, tag="mm")
                for kc in range(DC):
                    nc.tensor.matmul(ps_s, lhsT=ones[:, 0:P], rhs=src[:, kc, sl],
                                     start=(kc == 0), stop=(kc == DC - 1))
                for kc in range(DC):
                    nc.tensor.matmul(ps_q, lhsT=ones[:, 0:P], rhs=x2[:, kc, sl],
                                     start=(kc == 0), stop=(kc == DC - 1))
                m_t = lnpool.tile([P, NT], BF16, tag="m_t")
                q_t = lnpool.tile([P, NT], BF16, tag="q_t")
                nc.scalar.mul(out=m_t, in_=ps_s, mul=1.0 / D)
                nc.scalar.mul(out=q_t, in_=ps_q, mul=1.0 / D)
                var = lnpool.tile([P, NT], BF16, tag="var")
                nc.vector.scalar_tensor_tensor(
                    out=var, in0=m_t, scalar=1.0, in1=m_t,
                    op0=OP.bypass, op1=OP.mult)
                nc.vector.scalar_tensor_tensor(
                    out=var, in0=var, scalar=-1.0, in1=q_t,
                    op0=OP.mult, op1=OP.add)
                r16 = lnpool.tile([P, NT], BF16, tag="r16")
                rstd16(var, r16)
                nc.vector.tensor_copy(out=r_h[:, sl], in_=r16)
                nc.vector.scalar_tensor_tensor(
                    out=mr_h[:, sl], in0=m_t, scalar=1.0, in1=r16,
                    op0=OP.bypass, op1=OP.mult)
            g_sb = None
            if w_gains is not None:
                g_sb = spool.tile([P, 2, DC], F32, tag="g_sb")
                nc.sync.dma_start(
                    g_sb, w_gains[lidx, 2 * which : 2 * which + 2]
                    .rearrange("g (c p) -> p g c", p=P))
            for kc in range(DC):
                for tci in range(TC):
                    sl = slice(tci * NT, (tci + 1) * NT)
                    u = tpool.tile([P, NT], BF16, tag="ln_u")
                    nc.vector.scalar_tensor_tensor(
                        out=u, in0=src[:, kc, sl], scalar=1.0, in1=r_h[:, sl],
                        op0=OP.bypass, op1=OP.mult)
                    tgt = dst[:, kc, sl]
                    nc.vector.scalar_tensor_tensor(
                        out=tgt, in0=u, scalar=1.0, in1=mr_h[:, sl],
                        op0=OP.bypass, op1=OP.subtract)
                    if g_sb is not None:
                        nc.scalar.activation(
                            out=tgt, in_=tgt, func=AF.Identity,
                            bias=g_sb[:, 1, kc : kc + 1], scale=g_sb[:, 0, kc : kc + 1])
                    if out_stream is not None:
                        pool_o, dview = out_stream
                        of = pool_o.tile([P, NT], F32, tag="of")
                        nc.vector.scalar_tensor_tensor(
                            out=of, in0=u, scalar=1.0, in1=mr_h[:, sl],
                            op0=OP.bypass, op1=OP.subtract)
                        if g_sb is not None:
                            nc.scalar.activation(
                                out=of, in_=of, func=AF.Identity,
                                bias=g_sb[:, 1, kc : kc + 1],
                                scale=g_sb[:, 0, kc : kc + 1])
                        if fin_sb is not None:
                            nc.scalar.activation(
                                out=of, in_=of, func=AF.Identity,
                                bias=fin_sb[:, 1, kc : kc + 1],
                                scale=fin_sb[:, 0, kc : kc + 1])
                        nc.sync.dma_start(dview[:, kc, sl], of)

        def load_w(dram, lidx, tag):
            _, fin, fout = dram.shape
            wt = wpool.tile([P, fin // P, fout], BF16, tag=tag)
            nc.sync.dma_start(wt, dram[lidx].rearrange("(c p) f -> p c f", p=P))
            return wt

        bias_sb = [None]

        def bias_mm(ps, row, mslice, tcslice_n):
            """Add bias row (features mslice) into psum via a K=1 matmul."""
            if bias_sb[0] is None:
                return
            nc.tensor.matmul(
                ps, lhsT=bias_sb[0][0:1, row, mslice], rhs=ones[0:1, 0:tcslice_n],
                start=False, stop=True, skip_group_check=True)

        def emit_ffn(wa_d, wb_d, rows, lidx, src):
            w1 = load_w(wa_d, lidx, "w1")
            w2 = load_w(wb_d, lidx, "w2")
            for tci in range(TC):
                sl = slice(tci * NT, (tci + 1) * NT)
                acc = [psum.tile([P, NT], F32, tag="mm", name=f"acc{i}")
                       for i in range(DC)]
                for m in range(FC):
                    ph = psum.tile([P, NT], F32, tag="mm")
                    for kc in range(DC):
                        nc.tensor.matmul(
                            ph, lhsT=w1[:, kc, m * P : (m + 1) * P], rhs=src[:, kc, sl],
                            start=(kc == 0), stop=(kc == DC - 1 and not bias_gen))
                    bias_mm(ph, rows[0], slice(m * P, (m + 1) * P), NT)
                    hb = hpool.tile([P, NT], BF16, tag="h1")
                    nc.scalar.activation(out=hb, in_=ph, func=AF.Silu)
                    for dcc in range(DC):
                        nc.tensor.matmul(
                            acc[dcc], lhsT=w2[:, m, dcc * P : (dcc + 1) * P], rhs=hb,
                            start=(m == 0), stop=(m == FC - 1 and not bias_gen),
                            skip_group_check=True)
                for dcc in range(DC):
                    bias_mm(acc[dcc], rows[1], slice(dcc * P, (dcc + 1) * P), NT)
                    nc.vector.scalar_tensor_tensor(
                        out=x[:, dcc, sl], in0=acc[dcc], scalar=1.0,
                        in1=x[:, dcc, sl], op0=OP.bypass, op1=OP.add)

        for l in range(LAYERS):
            if bias_gen:
                bt = wpool.tile([1, 10, 2 * EC], BF16, tag="bias")
                nc.sync.dma_start(bt, w_bias[l])
                bias_sb[0] = bt

            # ===== FFN1 (half residual) =====
            if "f" in PHASES:
                if l == 0 or ln_gen:
                    emit_ln(x, xh, l, 0)
                    src1 = xh
                else:
                    src1 = x  # already unit-normalized by previous blk LN
                emit_ffn(w_f1a, w_f1b, (BR_F1B1, BR_F1B2), l, src1)

            # ===== local windowed MHSA =====
            if "a" in PHASES or "A" in PHASES:
                emit_ln(x, xh, l, 1)
                wq = load_w(w_q, l, "wq")
                wk = load_w(w_kk, l, "wk")
                wv = load_w(w_v, l, "wv")
                wo = load_w(w_o, l, "wo")
                for m in range(DC):
                    for tci in range(TC):
                        sl = slice(tci * NT, (tci + 1) * NT)
                        pq = psum.tile([P, NT], F32, tag="mm")
                        for kc in range(DC):
                            nc.tensor.matmul(
                                pq, lhsT=wq[:, kc, m * P : (m + 1) * P],
                                rhs=xh[:, kc, sl],
                                start=(kc == 0), stop=(kc == DC - 1 and not bias_gen))
                        bias_mm(pq, BR_Q, slice(m * P, (m + 1) * P), NT)
                        nc.scalar.copy(out=qT[:, m, sl], in_=pq)
                        pk = psum.tile([P, NT], F32, tag="mm")
                        for kc in range(DC):
                            nc.tensor.matmul(
                                pk, lhsT=wk[:, kc, m * P : (m + 1) * P],
                                rhs=xh[:, kc, sl],
                                start=(kc == 0), stop=(kc == DC - 1 and not bias_gen))
                        bias_mm(pk, BR_K, slice(m * P, (m + 1) * P), NT)
                        nc.scalar.copy(
                            out=kT[:, m, 32 + tci * NT : 32 + (tci + 1) * NT], in_=pk)
                for tb in range(TB):
                    pv = psum.tile([P, NT], F32, tag="mm")
                    for kc in range(DC):
                        nc.tensor.matmul(
                            pv, lhsT=xh[:, kc, tb * P : (tb + 1) * P],
                            rhs=wv[:, kc, 0:D],
                            start=(kc == 0), stop=(kc == DC - 1 and not bias_gen))
                    if bias_gen:
                        nc.tensor.matmul(
                            pv, lhsT=ones[0:1, 0:P], rhs=bias_sb[0][0:1, BR_V, 0:D],
                            start=False, stop=True, skip_group_check=True)
                    vstage = hpool.tile([P, D], BF16, tag="vstage")
                    nc.vector.tensor_copy(out=vstage, in_=pv)
                    nc.sync.dma_start(vS[32:128, tb, :], vstage[0:96, :])
                    nc.sync.dma_start(vS[0:32, tb + 1, :], vstage[96:128, :])
                for hp in range(DC):
                    for qb in range(TB):
                        band = bands["q0"] if qb == 0 else (
                            bands["q7"] if qb == TB - 1 else bands["mid"])
                        po = psum.tile([P, P], F32, tag="mm")
                        for hh in range(2):
                            pr = slice(hh * DH, (hh + 1) * DH)
                            ps_s = psum.tile([P, KW], F32, tag="mm")
                            nc.tensor.matmul(
                                ps_s, lhsT=qT[pr, hp, qb * P : (qb + 1) * P],
                                rhs=kT[pr, hp, qb * P : qb * P + KW],
                                start=True, stop=True)
                            at = atpool.tile([P, KW], BF16, tag="at")
                            nc.scalar.activation(out=at, in_=ps_s, func=AF.Exp)
                            lsum = smpool.tile([P, 1], F32, tag="l")
                            nc.vector.scalar_tensor_tensor(
                                out=at, in0=at, scalar=1.0, in1=band,
                                op0=OP.bypass, op1=OP.mult, accum_out=lsum)
                            rl = smpool.tile([P, 1], F32, tag="rl")
                            nc.vector.reciprocal(out=rl, in_=lsum)
                            nc.vector.tensor_scalar_mul(out=at, in0=at, scalar1=rl)
                            pt = psum.tile([P, KW], BF16, tag="mm")
                            nc.tensor.transpose(pt[:, 0:128], at[:, 0:128], ident)
                            nc.tensor.transpose(pt[0:64, 128:192],
                                                at[64:128, 128:192],
                                                ident[64:128, 64:128])
                            asb = atpool.tile([P, KW], BF16, tag="asb")
                            if hh == 0:
                                nc.vector.tensor_copy(out=asb, in_=pt)
                            else:
                                nc.scalar.copy(out=asb, in_=pt)
                            h = hp * 2 + hh
                            hc = slice(h * DH, (h + 1) * DH)
                            nc.tensor.matmul(
                                po[pr, :], lhsT=vS[:, qb, hc], rhs=asb[:, 0:128],
                                start=True, stop=False, skip_group_check=True)
                            nc.tensor.matmul(
                                po[pr, 64:128], lhsT=vS[0:64, qb + 1, hc],
                                rhs=asb[0:64, 128:192],
                                start=False, stop=True, skip_group_check=True)
                        nc.vector.tensor_copy(
                            out=oT[:, hp, qb * P : (qb + 1) * P], in_=po)
                if "A" not in PHASES:
                    for tci in range(TC):
                        sl = slice(tci * NT, (tci + 1) * NT)
                        for m in range(DC):
                            pp = psum.tile([P, NT], F32, tag="mm")
                            for kc in range(DC):
                                nc.tensor.matmul(
                                    pp, lhsT=wo[:, kc, m * P : (m + 1) * P],
                                    rhs=oT[:, kc, sl],
                                    start=(kc == 0), stop=(kc == DC - 1 and not bias_gen))
                            bias_mm(pp, BR_O, slice(m * P, (m + 1) * P), NT)
                            nc.vector.scalar_tensor_tensor(
                                out=x[:, m, sl], in0=pp, scalar=1.0, in1=x[:, m, sl],
                                op0=OP.bypass, op1=OP.add)

            # ===== convolution module =====
            if "c" in PHASES:
                emit_ln(x, xh, l, 2)
                p1 = load_w(w_p1, l, "w1")
                p2 = load_w(w_p2, l, "w2")
                dwt = wpool.tile([P, CC, KK], F32, tag="dw")
                nc.sync.dma_start(dwt, w_dw[l])
                dwb_sb = None
                if dwb_gen:
                    dwb_sb = wpool.tile([P, CC], F32, tag="dwb")
                    nc.sync.dma_start(dwb_sb, w_dwb[l])
                for mp in range(0, CC, 2):
                    pair = (mp, mp + 1)
                    cps = {}
                    strips = {}
                    for m in pair:
                        # diag strips for the 15 odd PE taps, DMA'd from the
                        # host-precomputed tensor
                        st = stpool.tile([P, len(PE_TAPS), P], FP16, tag="strip")
                        nc.sync.dma_start(st, w_strip[l, m])
                        strips[m] = st
                        cp = cpool.tile([P, KK - 1 + T], FP16, tag="cp")
                        nc.vector.memset(cp[:, 0 : KK // 2], 0.0)
                        nc.vector.memset(cp[:, KK // 2 + T :], 0.0)
                        cps[m] = cp
                        for tci in range(TC):
                            sl = slice(tci * NT, (tci + 1) * NT)
                            pb = psum.tile([P, NT], F32, tag="mm")
                            for kc in range(DC):
                                nc.tensor.matmul(
                                    pb, lhsT=p1[:, kc, EC + m * P : EC + (m + 1) * P],
                                    rhs=xh[:, kc, sl],
                                    start=(kc == 0), stop=(kc == DC - 1 and not bias_gen))
                            bias_mm(pb, BR_P1, slice(EC + m * P, EC + (m + 1) * P), NT)
                            tb_ = tpool.tile([P, NT], FP16, tag="th")
                            nc.scalar.activation(out=tb_, in_=pb, func=AF.Tanh, scale=0.5)
                            pa = psum.tile([P, NT], F32, tag="mm")
                            for kc in range(DC):
                                nc.tensor.matmul(
                                    pa, lhsT=p1[:, kc, m * P : (m + 1) * P],
                                    rhs=xh[:, kc, sl],
                                    start=(kc == 0), stop=(kc == DC - 1 and not bias_gen))
                            bias_mm(pa, BR_P1, slice(m * P, (m + 1) * P), NT)
                            nc.vector.scalar_tensor_tensor(
                                out=cp[:, KK // 2 + tci * NT : KK // 2 + (tci + 1) * NT],
                                in0=tb_, scalar=1.0, in1=pa, op0=OP.add, op1=OP.mult)
                    # PE: odd taps via strip matmuls, f32 psum accumulation
                    pcs = {}
                    for m in pair:
                        for tci in range(TC):
                            pc = psum.tile([P, NT], F32, tag="mm",
                                           name=f"pc{m}_{tci}")
                            for j, kk in enumerate(PE_TAPS):
                                nc.tensor.matmul(
                                    pc, lhsT=strips[m][:, j, :],
                                    rhs=cps[m][:, kk + tci * NT : kk + tci * NT + NT],
                                    start=(j == 0), stop=(j == len(PE_TAPS) - 1),
                                    skip_group_check=True)
                            pcs[(m, tci)] = pc
                    # DVE: even taps as fp16 acc chains (4 interleaved chains)
                    for j, kk in enumerate(DVE_TAPS):
                        for m in pair:
                            for tci in range(TC):
                                csl = c2[:, m, tci * NT : (tci + 1) * NT]
                                src_ = cps[m][:, kk + tci * NT : kk + tci * NT + NT]
                                if j == 0:
                                    nc.vector.tensor_scalar_mul(
                                        out=csl, in0=src_,
                                        scalar1=dwt[:, m, kk : kk + 1])
                                else:
                                    nc.vector.scalar_tensor_tensor(
                                        out=csl, in0=src_,
                                        scalar=dwt[:, m, kk : kk + 1], in1=csl,
                                        op0=OP.mult, op1=OP.add)
                    # combine PE + DVE halves (+ optional dw bias)
                    for m in pair:
                        for tci in range(TC):
                            csl = c2[:, m, tci * NT : (tci + 1) * NT]
                            dwb_s = dwb_sb[:, m : m + 1] if dwb_sb is not None else 0.0
                            nc.vector.scalar_tensor_tensor(
                                out=csl, in0=pcs[(m, tci)], scalar=dwb_s,
                                in1=csl, op0=OP.add, op1=OP.add)
                # GroupNorm(1 group over [EC, T]) stats
                parts = []
                for tci in range(TC):
                    sl = slice(tci * NT, (tci + 1) * NT)
                    ps_s = psum.tile([P, NT], F32, tag="mm")
                    ps_q = psum.tile([P, NT], F32, tag="mm")
                    for m in range(CC):
                        nc.tensor.matmul(ps_s, lhsT=onesh[:, 0:P], rhs=c2[:, m, sl],
                                         start=(m == 0), stop=(m == CC - 1))
                    for m in range(CC):
                        cs = hpool.tile([P, NT], FP16, tag="cs")
                        nc.scalar.square(out=cs, in_=c2[:, m, sl])
                        nc.tensor.matmul(ps_q, lhsT=onesh[:, 0:P], rhs=cs,
                                         start=(m == 0), stop=(m == CC - 1))
                    rs = smpool.tile([P, 1], F32, tag=f"gs{tci}")
                    rq = smpool.tile([P, 1], F32, tag=f"gq{tci}")
                    nc.vector.tensor_reduce(out=rs, in_=ps_s,
                                            axis=mybir.AxisListType.X, op=OP.add)
                    nc.vector.tensor_reduce(out=rq, in_=ps_q,
                                            axis=mybir.AxisListType.X, op=OP.add)
                    parts.append((rs, rq))
                gs = smpool.tile([P, 1], F32, tag="gsum")
                gq = smpool.tile([P, 1], F32, tag="gqsum")
                nc.vector.tensor_tensor(gs, parts[0][0], parts[1][0], OP.add)
                nc.vector.tensor_tensor(gq, parts[0][1], parts[1][1], OP.add)
                mg = smpool.tile([P, 1], F32, tag="mg")
                nc.vector.tensor_scalar_mul(out=mg, in0=gs, scalar1=1.0 / (EC * T))
                msqg = smpool.tile([P, 1], F32, tag="msqg")
                nc.vector.tensor_tensor(msqg, mg, mg, OP.mult)
                varg = smpool.tile([P, 1], F32, tag="varg")
                nc.vector.scalar_tensor_tensor(
                    out=varg, in0=gq, scalar=1.0 / (EC * T), in1=msqg,
                    op0=OP.mult, op1=OP.subtract)
                nc.vector.tensor_scalar_add(out=varg, in0=varg, scalar1=EPS)
                rg = smpool.tile([P, 1], F32, tag="rg")
                rstd32(varg, rg)
                # A = gn_g * r ; B = gn_b - m * A    (per-channel, [P, CC])
                gaff = spool.tile([P, 2, CC], F32, tag="gaff")
                nc.sync.dma_start(gaff, w_gn[l].rearrange("g (c p) -> p g c", p=P))
                a_t = spool.tile([P, CC], F32, tag="a_t")
                nc.vector.tensor_scalar_mul(out=a_t, in0=gaff[:, 0], scalar1=rg)
                mneg = smpool.tile([P, 1], F32, tag="mneg")
                nc.vector.tensor_scalar_mul(out=mneg, in0=mg, scalar1=-1.0)
                b_t = spool.tile([P, CC], F32, tag="b_t")
                nc.vector.scalar_tensor_tensor(
                    out=b_t, in0=a_t, scalar=mneg, in1=gaff[:, 1],
                    op0=OP.mult, op1=OP.add)
                for m in range(CC):
                    nc.scalar.activation(
                        out=c3[:, m], in_=c2[:, m], func=AF.Silu,
                        bias=b_t[:, m : m + 1], scale=a_t[:, m : m + 1])
                for tci in range(TC):
                    sl = slice(tci * NT, (tci + 1) * NT)
                    for dcc in range(DC):
                        pp = psum.tile([P, NT], F32, tag="mm")
                        for m in range(CC):
                            nc.tensor.matmul(
                                pp, lhsT=p2[:, m, dcc * P : (dcc + 1) * P],
                                rhs=c3[:, m, sl],
                                start=(m == 0), stop=(m == CC - 1 and not bias_gen))
                        bias_mm(pp, BR_P2, slice(dcc * P, (dcc + 1) * P), NT)
                        nc.vector.scalar_tensor_tensor(
                            out=x[:, dcc, sl], in0=pp, scalar=1.0, in1=x[:, dcc, sl],
                            op0=OP.bypass, op1=OP.add)

            # ===== FFN2 (half residual) =====
            if "2" in PHASES:
                emit_ln(x, xh, l, 3)
                emit_ffn(w_f2a, w_f2b, (BR_F2B1, BR_F2B2), l, xh)

            # ===== per-block LN =====
            if "b" in PHASES:
                if l == LAYERS - 1:
                    with tc.tile_pool(name="outp", bufs=3) as op_:
                        fin_sb = None
                        if w_fin is not None:
                            fin_sb = spool.tile([P, 2, DC], F32, tag="fin_sb")
                            nc.sync.dma_start(
                                fin_sb, w_fin.rearrange("g (c p) -> p g c", p=P))
                        emit_ln(x, x, l, 4,
                                out_stream=(op_, out_d.rearrange(
                                    "(c p) t -> p c t", p=P)),
                                fin_sb=fin_sb)
                else:
                    emit_ln(x, x, l, 4)

        if "b" not in PHASES or LAYERS == 0:
            # debug path: dump current x (or oT for 'A') as output
            with tc.tile_pool(name="outp", bufs=3) as op_:
                srcd = oT if "A" in PHASES else x
                dview = out_d.rearrange("(c p) t -> p c t", p=P)
                for kc in range(DC):
                    for tci in range(TC):
                        sl = slice(tci * NT, (tci + 1) * NT)
                        of = op_.tile([P, NT], F32, tag="of")
                        nc.vector.tensor_copy(out=of, in_=srcd[:, kc, sl])
                        nc.sync.dma_start(dview[:, kc, sl], of)

    nc.finalize()
    return nc


_PROG_CACHE = {}


def _get_program(flags):
    key = tuple(sorted(flags.items())) + (LAYERS, PHASES)
    if key not in _PROG_CACHE:
        _PROG_CACHE[key] = build_program(flags)
    return _PROG_CACHE[key]


def kernel(**inputs):
    global LAST_RESULT
    f32 = lambda a: np.asarray(a, dtype=np.float32)
    bf = lambda a: np.ascontiguousarray(f32(a).astype(ml_dtypes.bfloat16))
    x = f32(inputs["x"])                       # [B, T, D]

    def triv(names_vals):
        return all(bool(np.all(f32(inputs[n]) == v)) for n, v in names_vals)

    ln_trivial = triv(
        [(f"{p}_ln_g", 1.0) for p in ("ffn1", "attn", "conv", "ffn2", "blk")]
        + [(f"{p}_ln_b", 0.0) for p in ("ffn1", "attn", "conv", "ffn2", "blk")])
    final_trivial = triv([("final_ln_g", 1.0), ("final_ln_b", 0.0)])
    bias_trivial = triv([(n, 0.0) for n in (
        "ffn1_b1", "ffn1_b2", "qkv_b", "outp_b", "pw1_b", "pw2_b",
        "ffn2_b1", "ffn2_b2")])
    dwb_trivial = triv([("dw_b", 0.0)])
    flags = dict(ln_trivial=ln_trivial, final_trivial=final_trivial,
                 bias_trivial=bias_trivial, dwb_trivial=dwb_trivial)

    nc = _get_program(flags)

    qkv = f32(inputs["qkv_w"])                # [L, D, 3D]
    dwp = (f32(inputs["dw_w"]).reshape(L, EC, KK) * 0.5).reshape(L, CC, P, KK)
    dw = dwp.transpose(0, 2, 1, 3)            # [L, P, CC, K]
    npe = len(PE_TAPS)
    strips = np.zeros((L, CC, P, npe, P), dtype=np.float16)
    idx = np.arange(P)
    # strips[l, c, p, j, p] = dwp[l, c, p, PE_TAPS[j]]
    strips[:, :, idx, :, idx] = dwp[:, :, :, list(PE_TAPS)].transpose(
        2, 0, 1, 3).astype(np.float16)
    gn_aff = np.stack([f32(inputs["gn_g"]), f32(inputs["gn_b"])], axis=1)

    common = {
        "f1w1": bf(inputs["ffn1_w1"]),
        "f1w2": bf(f32(inputs["ffn1_w2"]) * 0.5),
        "f2w1": bf(inputs["ffn2_w1"]),
        "f2w2": bf(f32(inputs["ffn2_w2"]) * 0.5),
        "wq": bf(qkv[:, :, 0:D] * (DH ** -0.5)),
        "wk": bf(qkv[:, :, D : 2 * D]),
        "wv": bf(qkv[:, :, 2 * D : 3 * D]),
        "wo": bf(inputs["outp_w"]),
        "pw1": bf(inputs["pw1_w"]),
        "pw2": bf(inputs["pw2_w"]),
        "dw": np.ascontiguousarray(dw.astype(np.float32)),
        "strips": np.ascontiguousarray(strips),
        "gn_aff": np.ascontiguousarray(gn_aff.astype(np.float32)),
    }
    if not ln_trivial:
        rows = []
        for pfx in ("ffn1", "attn", "conv", "ffn2", "blk"):
            rows.append(f32(inputs[f"{pfx}_ln_g"]))
            rows.append(f32(inputs[f"{pfx}_ln_b"]))
        common["ln_gains"] = np.ascontiguousarray(
            np.stack(rows, axis=1).astype(np.float32))  # [L, 10, D]
    if not final_trivial:
        common["final_aff"] = np.ascontiguousarray(np.stack(
            [f32(inputs["final_ln_g"]), f32(inputs["final_ln_b"])]).astype(np.float32))
    if not bias_trivial:
        bias = np.zeros((L, 10, 2 * EC), np.float32)
        qb = f32(inputs["qkv_b"])
        bias[:, BR_F1B1, :FF] = f32(inputs["ffn1_b1"])
        bias[:, BR_F1B2, :D] = f32(inputs["ffn1_b2"]) * 0.5
        bias[:, BR_Q, :D] = qb[:, 0:D] * (DH ** -0.5)
        bias[:, BR_K, :D] = qb[:, D : 2 * D]
        bias[:, BR_V, :D] = qb[:, 2 * D : 3 * D]
        bias[:, BR_O, :D] = f32(inputs["outp_b"])
        bias[:, BR_P1, : 2 * EC] = f32(inputs["pw1_b"])
        bias[:, BR_P2, :D] = f32(inputs["pw2_b"])
        bias[:, BR_F2B1, :FF] = f32(inputs["ffn2_b1"])
        bias[:, BR_F2B2, :D] = f32(inputs["ffn2_b2"]) * 0.5
        common["biases"] = bf(bias)
    if not dwb_trivial:
        dwb = f32(inputs["dw_b"]).reshape(L, CC, P).transpose(0, 2, 1)
        common["dwb"] = np.ascontiguousarray(dwb.astype(np.float32))

    in_maps = []
    for c in range(B):
        m = dict(common)
        m["x_t"] = np.ascontiguousarray(x[c].T)   # [D, T] fp32
        in_maps.append(m)

    res = run_bass_kernel_spmd(
        nc, in_maps, core_ids=list(range(B)), trace=TRACE, **TRACE_KW)
    LAST_RESULT = res
    out = np.stack([r["out_t"].T for r in res.results]).astype(np.float32)
    return out


if __name__ == "__main__":
    rng = np.random.default_rng(0)
    ins = {"x": rng.standard_normal((B, T, D), dtype=np.float32)}
    # minimal smoke test requires full inputs; use test.py instead
    print("use test.py")
